# revision 57
# baseline (speedup 1.0000x reference)
"""GNN message-passing attention kernel for Trainium2 (Bass/Tile).

Problem: 3 iterations of masked single-head attention over 1024 independent
graphs (N=256 nodes, V=40 features, QK=50).

Sharding: data-parallel on the leading F axis -- 128 graphs per NeuronCore
across 8 cores.  Weights replicated.  Full inputs in, full output out.

Dataflow ("transposed-e" layout, gb=2 graphs per pipeline step, S streams
phase-interleaved in trace order so every engine always has independent
work queued):
  - Values carry an appended ones-column; transposed values vt then carry a
    ones-row, so the q/k biases ride inside the weight matmuls (fp32r fast
    PE path; fp32r matmuls/transposes must write PSUM partition 0).
  - One Tanh ACT per pair over the q|k PSUM block [50, 1024].
  - e^T[l, j] = k_l . q_j accumulated on top of MASKC*adjT (adjacency
    host-transposed, bf16; mask via a scaled-identity matmul):
    softmax mask becomes exp(e/s - 1000 + 1000*adj), no vector op.
  - One Exp ACT per pair produces num^T; nv[j, v] = sum_l num[j, l] v[l, v]
    computed directly off num^T (l already on partitions); the ones column
    makes column V the softmax row-sum.
  - Per-partition reciprocal + tensor_scalar normalize during the
    PSUM->SBUF move; rowsum*recip lands exactly 1.0, refreshing the
    ones-column for the next iteration for free.
"""

import math
import sys

import numpy as np

sys.path.insert(0, "/opt/trn_rl_repo")

import concourse.bass as bass  # noqa: E402
import concourse.mybir as mybir  # noqa: E402
import ml_dtypes  # noqa: E402
from concourse import bacc, tile  # noqa: E402
from concourse.bass_utils import run_bass_kernel_spmd  # noqa: E402
from concourse.masks import make_identity  # noqa: E402

# Problem constants (hardcoded per harness contract).
F, N, V, QK = 1024, 256, 40, 50
ITERS = 3
SCALE = math.sqrt(50.0)  # NUM_QK = 50
MASKC = 1000.0 * SCALE  # adj * MASKC accumulated into e; exp bias -1000
N_CORES = 8
G = F // N_CORES  # graphs per core
NC2 = N // 128  # 2 partition chunks of the node axis

F32 = mybir.dt.float32
F32R = mybir.dt.float32r  # fp32 data through the fast (replicated) PE path
BF16 = mybir.dt.bfloat16

DEFAULT_BUFS = dict(io=10, work=10, small=11, vnb=22, pmain=3, paux=2)


def build_nc(g_count=G, gb=2, streams=8, group=4, bufs=None):
    """Build the single-core Bass program (SPMD across 8 cores)."""
    B = dict(DEFAULT_BUFS)
    if bufs:
        B.update(bufs)
    streams = min(streams, g_count // gb)
    assert g_count % (gb * streams) == 0
    group = min(group, streams)
    nc = bacc.Bacc("TRN2", target_bir_lowering=False, debug=False)

    values_d = nc.dram_tensor("values", [g_count, N, V + 1], F32, kind="ExternalInput")
    adjt_d = nc.dram_tensor("adjt", [g_count, N, N], BF16, kind="ExternalInput")
    wq_d = nc.dram_tensor("wq_aug", [V + 1, QK], F32R, kind="ExternalInput")
    wk_d = nc.dram_tensor("wk_aug", [V + 1, QK], F32R, kind="ExternalInput")
    out_d = nc.dram_tensor("out", [g_count, N, V], F32, kind="ExternalOutput")

    with tile.TileContext(nc) as tc:
        with (
            tc.tile_pool(name="const", bufs=1) as constp,
            tc.tile_pool(name="io", bufs=B["io"]) as iop,
            tc.tile_pool(name="work", bufs=B["work"]) as workp,
            tc.tile_pool(name="small", bufs=B["small"]) as smallp,
            tc.tile_pool(name="pmain", bufs=B["pmain"], space="PSUM") as pmainp,
            tc.tile_pool(name="paux", bufs=B["paux"], space="PSUM") as pauxp,
        ):
            wq_sb = constp.tile([V + 1, QK], F32R)
            nc.sync.dma_start(wq_sb, wq_d[:, :])
            wk_sb = constp.tile([V + 1, QK], F32R)
            nc.sync.dma_start(wk_sb, wk_d[:, :])
            expbias_sb = constp.tile([128, 1], F32)
            nc.gpsimd.memset(expbias_sb, -1000.0)
            id_f32 = constp.tile([128, 128], F32)
            make_identity(nc, id_f32)
            idm_bf = constp.tile([128, 128], BF16)
            nc.vector.tensor_copy(idm_bf, id_f32)

            class Stream:
                pass

            def phase_load(st, g0):
                st.prev_g0 = getattr(st, "g0", None)
                st.prev_vn = getattr(st, "vn", None)
                st.g0 = g0
                gsl = slice(g0, g0 + gb)
                st.vn = iop.tile([128, gb, NC2, V + 1], F32, tag="vn", bufs=B["vnb"])
                nc.sync.dma_start(
                    st.vn,
                    values_d[gsl, :, :].rearrange("g (c p) v -> p g c v", c=NC2),
                )
                st.adjt = iop.tile([128, gb, NC2, N], BF16, tag="adj")
                nc.sync.dma_start(
                    st.adjt, adjt_d[gsl, :, :].rearrange("g (c p) j -> p g c j", c=NC2)
                )

            def phase_vt0(st):
                psum_vt = pauxp.tile([V + 1, gb * N], F32, tag="paux")
                for g in range(gb):
                    for c in range(NC2):
                        nc.tensor.transpose(
                            psum_vt[:, N * g + 128 * c : N * g + 128 * (c + 1)],
                            st.vn[:, g, c, :],
                            id_f32,
                        )
                st.vt = smallp.tile([V + 1, gb * N], F32R, tag="vt")
                nc.vector.tensor_copy(st.vt, psum_vt)

            def phase_qk(st):
                # [50, (qk-half, g, j)]: q in bank 0, k in bank 1.
                # Bias rides the vt ones-row (weights row V).
                st.psum_qk = pmainp.tile([QK, 2 * gb * N], F32, tag="pmain")
                nc.tensor.matmul(st.psum_qk[:, 0 : gb * N], wq_sb, st.vt)
                nc.tensor.matmul(st.psum_qk[:, gb * N : 2 * gb * N], wk_sb, st.vt)

            def phase_tanh(st):
                st.qk = workp.tile([QK, 2 * gb * N], F32R, tag="qk")
                nc.scalar.activation(
                    st.qk, st.psum_qk, mybir.ActivationFunctionType.Tanh
                )
                st.psum_qk = None

            def phase_mask(st):
                # graph 0: additive mask preloaded into PSUM on PE;
                # graph 1: added by DVE after its score matmuls (phase_masktt)
                st.psum_e = pmainp.tile([128, gb, NC2 * N], F32, tag="pmain", name="pe")
                nc.tensor.matmul(
                    st.psum_e[:, 0, :],
                    idm_bf,
                    st.adjt[:, 0, :, :].rearrange("p c j -> p (c j)"),
                    start=True,
                    stop=False,
                    skip_group_check=True,
                )

            def phase_et(st):
                for g in range(gb):
                    for lc in range(NC2):
                        nc.tensor.matmul(
                            st.psum_e[:, g, N * lc : N * (lc + 1)],
                            st.qk[:, gb * N + N * g + 128 * lc : gb * N + N * g + 128 * (lc + 1)],
                            st.qk[:, N * g : N * (g + 1)],
                            start=(g > 0),
                            stop=True,
                            skip_group_check=True,
                        )

            def phase_masktt(st):
                nc.vector.tensor_add(
                    st.psum_e[:, 1, :],
                    st.psum_e[:, 1, :],
                    st.adjt[:, 1, :, :].rearrange("p c j -> p (c j)"),
                )

            def phase_exp(st):
                st.numt = workp.tile([128, gb, NC2 * N], F32, tag="numt")
                nc.scalar.activation(
                    st.numt,
                    st.psum_e,
                    mybir.ActivationFunctionType.Exp,
                    bias=expbias_sb,
                    scale=1.0 / SCALE,
                )
                st.psum_e = None

            def phase_nv(st):
                # nv[j, v] = sum_l num[j, l] v[l, v], directly off numT
                # (l already on partitions); the vn ones-column makes col V
                # the softmax row-sum.
                st.psum_nv = pauxp.tile([128, gb, NC2, V + 1], F32, tag="paux")
                for g in range(gb):
                    for jc in range(NC2):
                        for lc in range(NC2):
                            nc.tensor.matmul(
                                st.psum_nv[:, g, jc, :],
                                st.numt[:, g, N * lc + 128 * jc : N * lc + 128 * jc + 128],
                                st.vn[:, g, lc, :],
                                start=(lc == 0),
                                stop=(lc == NC2 - 1),
                            )
                st.numt = None

            def phase_norm(st):
                recip = smallp.tile([128, gb, NC2], F32, tag="recip")
                nc.vector.reciprocal(recip, st.psum_nv[:, :, :, V])
                st.vn = iop.tile([128, gb, NC2, V + 1], F32, tag="vn", bufs=B["vnb"])
                for g in range(gb):
                    for jc in range(NC2):
                        nc.vector.tensor_scalar_mul(
                            st.vn[:, g, jc, :],
                            st.psum_nv[:, g, jc, :],
                            recip[:, g, jc : jc + 1],
                        )
                st.psum_nv = None

            def phase_vt(st):
                psum_vt = pauxp.tile([V + 1, gb * N], F32, tag="paux")
                for g in range(gb):
                    for jc in range(NC2):
                        nc.tensor.transpose(
                            psum_vt[:, N * g + 128 * jc : N * g + 128 * (jc + 1)],
                            st.vn[:, g, jc, :],
                            id_f32,
                        )
                st.vt = smallp.tile([V + 1, gb * N], F32R, tag="vt")
                nc.vector.tensor_copy(st.vt, psum_vt)

            def phase_store_prev(st):
                # SWDGE (gpsimd) queue: keeps result stores out of the SP
                # FIFO so the next round's loads always prefetch early.
                gsl = slice(st.prev_g0, st.prev_g0 + gb)
                nc.gpsimd.dma_start(
                    out_d[gsl, :, :].rearrange("g (c p) v -> p g c v", c=NC2),
                    st.prev_vn[:, :, :, 0:V],
                )

            sts = [Stream() for _ in range(streams)]
            for _i, _st in enumerate(sts):
                _st.sid = _i
            grps = [sts[i : i + group] for i in range(0, streams, group)]

            def run_iter(grp, t):
                for st in grp:
                    phase_qk(st)
                for st in grp:
                    phase_mask(st)
                for st in grp:
                    phase_tanh(st)
                for st in grp:
                    phase_et(st)
                for st in grp:
                    phase_masktt(st)
                for st in grp:
                    phase_exp(st)
                for st in grp:
                    phase_nv(st)
                for st in grp:
                    phase_norm(st)
                if t < ITERS - 1:
                    for st in grp:
                        phase_vt(st)

            # Groups round-robin per iteration so one group's next phase
            # fills the pipeline while the other finishes; the previous
            # round's store and the next round's load ride inside the
            # rotation so round boundaries never resynchronize the streams.
            rounds = g_count // (gb * streams)
            for r in range(rounds):
                for grp in grps:
                    for st in grp:
                        phase_load(st, gb * (r * streams + st.sid))
                for grp in grps:
                    for st in grp:
                        if r > 0:
                            phase_store_prev(st)
                    for st in grp:
                        phase_vt0(st)
                for t in range(ITERS):
                    for grp in grps:
                        run_iter(grp, t)
            for grp in grps:
                for st in grp:
                    st.prev_g0, st.prev_vn = st.g0, st.vn
                    phase_store_prev(st)

    nc.compile()
    return nc


_NC_CACHE = None


def _get_nc():
    global _NC_CACHE
    if _NC_CACHE is None:
        _NC_CACHE = build_nc()
    return _NC_CACHE


def _make_in_maps(values, adjacency_matrix, Wq, bq, Wk, bk):
    values = np.asarray(values, dtype=np.float32).reshape(F, N, V)
    values = np.concatenate([values, np.ones((F, N, 1), np.float32)], axis=2)
    adj = np.asarray(adjacency_matrix, dtype=np.float32).reshape(F, N, N)
    adjt = (np.ascontiguousarray(adj.transpose(0, 2, 1)) * MASKC).astype(ml_dtypes.bfloat16)

    def _aug(W, b):
        aug = np.zeros((V + 1, QK), np.float32)
        aug[0:V] = np.asarray(W, np.float32).T
        aug[V] = np.asarray(b, np.float32)
        return aug

    wq_aug = _aug(Wq, bq)
    wk_aug = _aug(Wk, bk)
    in_maps = []
    for i in range(N_CORES):
        sl = slice(i * G, (i + 1) * G)
        in_maps.append(
            {
                "values": np.ascontiguousarray(values[sl]),
                "adjt": np.ascontiguousarray(adjt[sl]),
                "wq_aug": wq_aug,
                "wk_aug": wk_aug,
            }
        )
    return in_maps


def run_spmd(values, adjacency_matrix, Wq, bq, Wk, bk, trace=False):
    """Run on 8 cores; returns (full_output, BassKernelResults)."""
    nc = _get_nc()
    in_maps = _make_in_maps(values, adjacency_matrix, Wq, bq, Wk, bk)
    res = run_bass_kernel_spmd(nc, in_maps, core_ids=list(range(N_CORES)), trace=trace)
    outs = [np.asarray(r["out"]) for r in res.results]
    full = np.concatenate(outs, axis=0).reshape(F, 1, N, V).astype(np.float32)
    return full, res


def kernel(**inputs):
    out, _ = run_spmd(
        inputs["values"],
        inputs["adjacency_matrix"],
        inputs["Wq"],
        inputs["bq"],
        inputs["Wk"],
        inputs["bk"],
    )
    return out
